# revision 9
# baseline (speedup 1.0000x reference)
"""GAT attention-score kernel for Trainium2 (8 NeuronCores, SPMD).

Computes e = LeakyReLU(Wx_i @ a[:D] + Wx_j @ a[D:], slope=0.2) for
E=640000 edges, D=128, sharded over 8 cores along the edge dimension
(a is replicated to every core).

Per-core layout (E_CORE = 80000 edges):
  - partition p owns edges [p*EPP, (p+1)*EPP) of the core's shard
  - T tiles of C edges/partition; each tile is one SBUF tensor
    [128, 2, C, 128] (Wx_i half + Wx_j half, two contiguous DMAs)
  - the attention vector a is broadcast once to all partitions
    (a_sb [128, 2, 128])
  - per-tile compute, one of three flavors (tile_plan), so the dot
    products can be spread over VectorE / GPSIMD / ScalarE and the
    kernel stays DMA-bound.  Every flavor first computes
    prod = rec * a (broadcast along edges), then reduces each edge's
    256 products:
      'W': VectorE mult, VectorE tensor_reduce(axis=XY) per tile
      'A': VectorE mult, ScalarE activation(Copy, accum_out) per edge
      'B': GPSIMD mult, ScalarE activation(Copy, accum_out) per edge
  - LeakyReLU on the [128, C] accumulator, store via a separate DMA ring.
"""

import sys

if "/opt/trn_rl_repo" not in sys.path:
    sys.path.insert(0, "/opt/trn_rl_repo")

from contextlib import ExitStack

import numpy as np

import concourse.bass as bass
import concourse.bacc as bacc
import concourse.mybir as mybir
import concourse.tile as tile
from concourse.bass_utils import run_bass_kernel_spmd

N_CORES = 8
E = 640000
D = 128
REC = 2 * D
E_CORE = E // N_CORES  # 80000
P = 128
EPP = E_CORE // P  # 625 edges per partition
NEG_SLOPE = 0.2
F32 = mybir.dt.float32
MULT = mybir.AluOpType.mult
ADD = mybir.AluOpType.add
MAX = mybir.AluOpType.max

TILE_PLAN = "WAB"  # cycled over tiles


def _bcast_free(ap: bass.AP, count: int, axis: int) -> bass.AP:
    """Insert a stride-0 free dim of `count` at free-axis position `axis`."""
    dims = list(ap.ap)
    dims.insert(1 + axis, [0, count])
    return bass.AP(tensor=ap.tensor, offset=ap.offset, ap=dims)


def build_program(
    epp: int = EPP, c: int = 25, bufs: int = 3, plan: str = TILE_PLAN
) -> bass.Bass:
    """Build the per-core Bass program for `epp` edges per partition."""
    assert epp % c == 0
    t_tiles = epp // c
    e_core = P * epp

    nc = bacc.Bacc()
    wi_d = nc.dram_tensor("Wx_i", [e_core, D], F32, kind="ExternalInput")
    wj_d = nc.dram_tensor("Wx_j", [e_core, D], F32, kind="ExternalInput")
    a_d = nc.dram_tensor("a", [REC], F32, kind="ExternalInput")
    out_d = nc.dram_tensor("out", [e_core], F32, kind="ExternalOutput")

    wi = wi_d[:].rearrange("(p n) d -> p n d", p=P)  # [128, epp, 128]
    wj = wj_d[:].rearrange("(p n) d -> p n d", p=P)
    out_r = out_d[:].rearrange("(p t c) -> p t c", p=P, t=t_tiles, c=c)

    with tile.TileContext(nc) as tc, ExitStack() as ctx:
        const_pool = ctx.enter_context(tc.tile_pool(name="const", bufs=1))
        in_pool = ctx.enter_context(tc.tile_pool(name="inp", bufs=bufs))
        prod_pool = ctx.enter_context(tc.tile_pool(name="prod", bufs=2))
        acc_pool = ctx.enter_context(tc.tile_pool(name="acc", bufs=4))
        res_pool = ctx.enter_context(tc.tile_pool(name="res", bufs=4))

        # attention vector broadcast to all 128 partitions: [128, 2, 128]
        a_sb = const_pool.tile([P, 2, D], F32)
        a_ap = a_d[:]
        a_bcast = bass.AP(
            tensor=a_ap.tensor, offset=a_ap.offset, ap=[[0, P]] + list(a_ap.ap)
        )
        nc.gpsimd.dma_start(out=a_sb[:].rearrange("p a d -> p (a d)"), in_=a_bcast)

        # stride-0 garbage sink for the elementwise out of ScalarE accums
        sink_s = const_pool.tile([P, 1], F32)

        for t in range(t_tiles):
            kind = plan[t % len(plan)]
            rec = in_pool.tile([P, 2, c, D], F32, tag="rec")
            nc.sync.dma_start(out=rec[:, 0, :, :], in_=wi[:, t * c : (t + 1) * c, :])
            nc.sync.dma_start(out=rec[:, 1, :, :], in_=wj[:, t * c : (t + 1) * c, :])

            acc = acc_pool.tile([P, c], F32, tag=f"acc_{kind}")

            prod = prod_pool.tile([P, 2, c, D], F32, tag="prod")
            # a_sb [P,2,D] viewed as [P,2,c,D] with stride-0 over c
            a_view = _bcast_free(a_sb[:], c, axis=1)
            eng = nc.gpsimd if kind == "B" else nc.vector
            eng.tensor_tensor(out=prod[:], in0=rec[:], in1=a_view, op=MULT)

            if kind == "W":
                pv = prod[:].rearrange("p m c d -> p c m d")
                nc.vector.tensor_reduce(
                    out=acc[:], in_=pv, axis=mybir.AxisListType.XY, op=ADD
                )
            else:  # 'A' / 'B': ScalarE accumulates per edge
                for cc in range(c):
                    in_ = prod[:, :, cc, :]
                    nc.scalar.activation(
                        out=sink_s[:].broadcast_to(in_.shape),
                        in_=in_,
                        func=mybir.ActivationFunctionType.Copy,
                        accum_out=acc[:, cc : cc + 1],
                    )

            res = res_pool.tile([P, c], F32, tag=f"res_{kind}")
            # leaky_relu(x) = max(0.2*x, x) on VectorE
            nc.vector.scalar_tensor_tensor(
                out=res[:], in0=acc[:], scalar=NEG_SLOPE, in1=acc[:],
                op0=MULT, op1=MAX,
            )
            nc.scalar.dma_start(out=out_r[:, t, :], in_=res[:])

    nc.compile()
    return nc


_CACHED_NC = None


def kernel(Wx_i: np.ndarray, Wx_j: np.ndarray, a: np.ndarray) -> np.ndarray:
    global _CACHED_NC
    if _CACHED_NC is None:
        _CACHED_NC = build_program()
    nc = _CACHED_NC

    Wx_i = np.ascontiguousarray(np.asarray(Wx_i, dtype=np.float32))
    Wx_j = np.ascontiguousarray(np.asarray(Wx_j, dtype=np.float32))
    a = np.ascontiguousarray(np.asarray(a, dtype=np.float32))

    in_maps = []
    for i in range(N_CORES):
        sl = slice(i * E_CORE, (i + 1) * E_CORE)
        in_maps.append(
            {
                "Wx_i": np.ascontiguousarray(Wx_i[sl]),
                "Wx_j": np.ascontiguousarray(Wx_j[sl]),
                "a": a,
            }
        )

    r = run_bass_kernel_spmd(nc, in_maps, core_ids=list(range(N_CORES)))
    return np.concatenate([m["out"] for m in r.results])


# revision 10
# speedup vs baseline: 1.1218x; 1.1218x over previous
"""GAT attention-score kernel for Trainium2 (8 NeuronCores, SPMD).

Computes e = LeakyReLU(Wx_i @ a[:D] + Wx_j @ a[D:], slope=0.2) for
E=640000 edges, D=128, sharded over 8 cores along the edge dimension
(a is replicated to every core).

Per-core layout (E_CORE = 80000 edges):
  - partition p owns edges [p*EPP, (p+1)*EPP) of the core's shard
  - T tiles of C edges/partition; each tile is one SBUF tensor
    [128, 2, C, 128] (Wx_i half + Wx_j half, one contiguous DMA each,
    split across the two HWDGE rings: SP for Wx_i, ACT for Wx_j)
  - the attention vector a is broadcast once to all partitions
    (a_sb [128, 2, 128])
  - per-tile compute = elementwise multiply by a (broadcast over edges)
    into a prod tile, then a per-edge reduction of the 256 products.
    The two passes are spread over three engines via the tile plan:
      'W': VectorE mult, VectorE tensor_reduce(axis=XY)
      'A': VectorE mult, ScalarE activation(Copy, accum_out) per edge
      'B': GPSIMD  mult, ScalarE activation(Copy, accum_out) per edge
      'X': GPSIMD  mult, VectorE tensor_reduce(axis=XY)
  - LeakyReLU per tile on VectorE (tensor_scalar + tensor_tensor max)
    into a [128, EPP] result buffer; one store DMA at the end.
"""

import sys

if "/opt/trn_rl_repo" not in sys.path:
    sys.path.insert(0, "/opt/trn_rl_repo")

from contextlib import ExitStack

import numpy as np

import concourse.bass as bass
import concourse.bacc as bacc
import concourse.mybir as mybir
import concourse.tile as tile
from concourse.bass_utils import run_bass_kernel_spmd

N_CORES = 8
E = 640000
D = 128
REC = 2 * D
E_CORE = E // N_CORES  # 80000
P = 128
EPP = E_CORE // P  # 625 edges per partition
NEG_SLOPE = 0.2
F32 = mybir.dt.float32
MULT = mybir.AluOpType.mult
ADD = mybir.AluOpType.add
MAX = mybir.AluOpType.max

TILE_PLAN = "WAXB"  # cycled over tiles


def _bcast_free(ap: bass.AP, count: int, axis: int) -> bass.AP:
    """Insert a stride-0 free dim of `count` at free-axis position `axis`."""
    dims = list(ap.ap)
    dims.insert(1 + axis, [0, count])
    return bass.AP(tensor=ap.tensor, offset=ap.offset, ap=dims)


def build_program(
    epp: int = EPP, c: int = 25, bufs: int = 3, plan: str = TILE_PLAN
) -> bass.Bass:
    """Build the per-core Bass program for `epp` edges per partition."""
    assert epp % c == 0
    t_tiles = epp // c
    e_core = P * epp

    nc = bacc.Bacc()
    wi_d = nc.dram_tensor("Wx_i", [e_core, D], F32, kind="ExternalInput")
    wj_d = nc.dram_tensor("Wx_j", [e_core, D], F32, kind="ExternalInput")
    a_d = nc.dram_tensor("a", [REC], F32, kind="ExternalInput")
    out_d = nc.dram_tensor("out", [e_core], F32, kind="ExternalOutput")

    wi = wi_d[:].rearrange("(p n) d -> p n d", p=P)  # [128, epp, 128]
    wj = wj_d[:].rearrange("(p n) d -> p n d", p=P)
    out_r = out_d[:].rearrange("(p n) -> p n", p=P)  # [128, epp]

    with tile.TileContext(nc) as tc, ExitStack() as ctx:
        const_pool = ctx.enter_context(tc.tile_pool(name="const", bufs=1))
        in_pool = ctx.enter_context(tc.tile_pool(name="inp", bufs=bufs))
        prod_pool = ctx.enter_context(tc.tile_pool(name="prod", bufs=3))
        acc_pool = ctx.enter_context(tc.tile_pool(name="acc", bufs=4))
        res_pool = ctx.enter_context(tc.tile_pool(name="res", bufs=1))

        # attention vector broadcast to all 128 partitions: [128, 2, 128]
        a_sb = const_pool.tile([P, 2, D], F32)
        a_ap = a_d[:]
        a_bcast = bass.AP(
            tensor=a_ap.tensor, offset=a_ap.offset, ap=[[0, P]] + list(a_ap.ap)
        )
        nc.gpsimd.dma_start(out=a_sb[:].rearrange("p a d -> p (a d)"), in_=a_bcast)

        # stride-0 garbage sink for the elementwise out of ScalarE accums
        sink_s = const_pool.tile([P, 1], F32)
        # scratch for the 0.2*x half of the leaky relu
        scaled = const_pool.tile([P, c], F32)

        res = res_pool.tile([P, epp], F32)

        for t in range(t_tiles):
            kind = plan[t % len(plan)]
            rec = in_pool.tile([P, 2, c, D], F32, tag="rec")
            nc.sync.dma_start(out=rec[:, 0, :, :], in_=wi[:, t * c : (t + 1) * c, :])
            nc.scalar.dma_start(out=rec[:, 1, :, :], in_=wj[:, t * c : (t + 1) * c, :])

            acc = acc_pool.tile([P, c], F32, tag=f"acc_{kind}")

            prod = prod_pool.tile([P, 2, c, D], F32, tag="prod")
            # a_sb [P,2,D] viewed as [P,2,c,D] with stride-0 over c
            a_view = _bcast_free(a_sb[:], c, axis=1)
            eng = nc.gpsimd if kind in "BX" else nc.vector
            eng.tensor_tensor(out=prod[:], in0=rec[:], in1=a_view, op=MULT)

            if kind in "WX":
                pv = prod[:].rearrange("p m c d -> p c m d")
                nc.vector.tensor_reduce(
                    out=acc[:], in_=pv, axis=mybir.AxisListType.XY, op=ADD
                )
            else:  # 'A' / 'B': ScalarE accumulates per edge
                for cc in range(c):
                    in_ = prod[:, :, cc, :]
                    nc.scalar.activation(
                        out=sink_s[:].broadcast_to(in_.shape),
                        in_=in_,
                        func=mybir.ActivationFunctionType.Copy,
                        accum_out=acc[:, cc : cc + 1],
                    )

            # leaky_relu(x) = max(0.2*x, x), standard DVE ops
            rblk = res[:, t * c : (t + 1) * c]
            nc.vector.tensor_scalar_mul(scaled[:], acc[:], NEG_SLOPE)
            nc.vector.tensor_tensor(out=rblk, in0=acc[:], in1=scaled[:], op=MAX)

        nc.sync.dma_start(out=out_r[:, :], in_=res[:])

    nc.compile()
    return nc


_CACHED_NC = None


def kernel(Wx_i: np.ndarray, Wx_j: np.ndarray, a: np.ndarray) -> np.ndarray:
    global _CACHED_NC
    if _CACHED_NC is None:
        _CACHED_NC = build_program()
    nc = _CACHED_NC

    Wx_i = np.ascontiguousarray(np.asarray(Wx_i, dtype=np.float32))
    Wx_j = np.ascontiguousarray(np.asarray(Wx_j, dtype=np.float32))
    a = np.ascontiguousarray(np.asarray(a, dtype=np.float32))

    in_maps = []
    for i in range(N_CORES):
        sl = slice(i * E_CORE, (i + 1) * E_CORE)
        in_maps.append(
            {
                "Wx_i": np.ascontiguousarray(Wx_i[sl]),
                "Wx_j": np.ascontiguousarray(Wx_j[sl]),
                "a": a,
            }
        )

    r = run_bass_kernel_spmd(nc, in_maps, core_ids=list(range(N_CORES)))
    return np.concatenate([m["out"] for m in r.results])
